# revision 12
# baseline (speedup 1.0000x reference)
"""Trainium2 Bass kernel for a DecoderRNN (embedding -> 24-step LSTM -> vocab projection).

Shapes (hardcoded): B=128, T=24, H=E=1024, V=32000, 8 NeuronCores.

Sharding:
  - input projection Xp = X @ W_ih^T + b: sharded over steps (3 per core),
    assembled on every core via 3 pipelined AllGathers;
  - LSTM recurrence: replicated full-batch on every core (PE matmul wall-time
    is independent of M<=128, so a batch shard would cost the same);
  - output projection W_out: sharded along vocab, 4000 columns per core.
All matmuls run in float32r (full-rate, ~1e-4 rel err; true fp32 is 4x slower).
"""

import numpy as np

import concourse.bass as bass
import concourse.tile as tile
import concourse.mybir as mybir
from concourse import bacc
from concourse.bass_utils import run_bass_kernel_spmd

B, T = 128, 24
H, E, V = 1024, 1024, 32000
NCORES = 8
TSH = T // NCORES          # 3 Xp step-tiles per core
VSH = V // NCORES          # 4000 vocab columns per core
VT = 500                   # projection N-tile (8 per core)
KT = H // 128              # 8 contraction chunks
NT4H = (4 * H) // 512      # 8 gate N-tiles of 512

F32 = mybir.dt.float32
F32R = mybir.dt.float32r
I32 = mybir.dt.int32

_CACHE = {}


def _phase_a(nc, tc, tensors):
    """Local Xp step-tiles (j=0..2 -> step c+8j) + AllGather across cores.

    Returns the 3 shared gathered DRAM tiles; gathered tile j holds steps
    [8j, 8j+8) as [rank, 128, 4096] blocks (rank r = step 8j + r)."""
    emb_c, caps_l, w_ihT, gbias, ident, dram = tensors
    xp_g = []
    with tc.tile_pool(name="a_w", bufs=1) as a_w, \
         tc.tile_pool(name="a_x", bufs=2) as a_x, \
         tc.tile_pool(name="a_xt", bufs=2) as a_xt, \
         tc.tile_pool(name="a_sb", bufs=4) as a_sb, \
         tc.tile_pool(name="a_ps", bufs=3, space="PSUM") as a_ps, \
         tc.tile_pool(name="a_tr", bufs=2, space="PSUM") as a_tr:
        wih = a_w.tile([128, KT, 4 * H], F32R)
        nc.sync.dma_start(wih[:], w_ihT[:])
        gb = a_w.tile([128, 4 * H], F32)
        nc.gpsimd.dma_start(out=gb[:], in_=gbias[None, :].to_broadcast([128, 4 * H]))
        idt = a_w.tile([128, 128], F32)
        nc.sync.dma_start(idt[:], ident[:])
        capst = a_w.tile([128, TSH], I32)
        nc.sync.dma_start(capst[:], caps_l[:])

        for j in range(TSH):
            bounce_in = dram.tile([128, 4 * H], F32R, tag=f"agin{j}")
            x_t = a_x.tile([128, E], F32, tag="x")
            nc.gpsimd.indirect_dma_start(
                out=x_t[:], out_offset=None, in_=emb_c[:],
                in_offset=bass.IndirectOffsetOnAxis(ap=capst[:, j:j + 1], axis=0))
            xt_T = a_xt.tile([128, KT, 128], F32R, tag="xt")
            for e in range(KT):
                ptr = a_tr.tile([128, 128], F32, tag="tr")
                nc.tensor.transpose(ptr[:], x_t[:, e * 128:(e + 1) * 128], idt[:])
                nc.vector.tensor_copy(xt_T[:, e, :], ptr[:])
            for n in range(NT4H):
                ns = slice(n * 512, (n + 1) * 512)
                ps = a_ps.tile([128, 512], F32, tag="ps")
                for k in range(KT):
                    nc.tensor.matmul(ps[:], xt_T[:, k, :], wih[:, k, ns],
                                     start=(k == 0), stop=(k == KT - 1))
                xp_sb = a_sb.tile([128, 512], F32R, tag="xp")
                nc.vector.tensor_add(xp_sb[:], ps[:], gb[:, ns])
                nc.sync.dma_start(bounce_in[:, ns], xp_sb[:])
            g = dram.tile([NCORES, 128, 4 * H], F32R, tag=f"agout{j}",
                          addr_space="Shared")
            cc = nc.gpsimd.collective_compute(
                "AllGather", mybir.AluOpType.bypass,
                ins=[bounce_in.opt()], outs=[g.opt()],
                replica_groups=[list(range(NCORES))])
            xp_g.append((g, cc))
    return xp_g


def _phase_b(nc, tc, tensors):
    """24 serial LSTM steps; h^T history to DRAM."""
    w_hhT, feats, h0T, ident, identr, xp_g, hT_dram = tensors
    with tc.tile_pool(name="b_w", bufs=1) as b_w, \
         tc.tile_pool(name="b_xp", bufs=4) as b_xp, \
         tc.tile_pool(name="b_act", bufs=1) as b_act, \
         tc.tile_pool(name="b_tmp", bufs=2) as b_tmp, \
         tc.tile_pool(name="b_ps", bufs=4, space="PSUM") as b_ps, \
         tc.tile_pool(name="b_tr", bufs=2, space="PSUM") as b_tr:
        whh = b_w.tile([128, KT, 4 * H], F32R)
        nc.sync.dma_start(whh[:], w_hhT[:])
        idt = b_w.tile([128, 128], F32)
        nc.sync.dma_start(idt[:], ident[:])
        idr = b_w.tile([128, 128], F32R)
        nc.sync.dma_start(idr[:], identr[:])
        c_st = b_w.tile([128, H], F32)
        nc.sync.dma_start(c_st[:], feats[:])
        tnh = b_w.tile([128, H], F32)
        h_t = b_w.tile([128, H], F32)
        # hT double-buffered across steps: gate matmuls of step t read h_{t-1}^T
        # from one buffer while the new h_t^T transposes land in the other.
        hT_a = b_w.tile([128, KT, 128], F32R, tag="hT0")
        hT_b = b_w.tile([128, KT, 128], F32R, tag="hT1")
        hT_bufs = [hT_a, hT_b]
        nc.sync.dma_start(hT_bufs[0][:], h0T[:])

        # gate cols: i [0,1024) f [1024,2048) g [2048,3072) o [3072,4096)
        ACT_FN = {0: "Sigmoid", 1: "Sigmoid", 2: "Sigmoid", 3: "Sigmoid",
                  4: "Tanh", 5: "Tanh", 6: "Sigmoid", 7: "Sigmoid"}
        # order so that c/tanh(c) halves are ready before the o tiles land:
        # [i0 g0 f0] -> c half 0, [i1 g1 f1] -> c half 1, then o halves.
        N_ORDER = [0, 4, 2, 1, 5, 3, 6, 7]

        def gate_mms(t, n, a_t, hT_src):
            ns = slice(n * 512, (n + 1) * 512)
            g, cc = xp_g[t // 8]
            xp_n = b_xp.tile([128, 512], F32R, tag="xpn")
            dma = nc.sync.dma_start(xp_n[:], g[t % 8, :, ns])
            # Tile does not order reads of the AllGather output after the
            # collective on its own; pin the edge explicitly.
            tile.add_dep_helper(dma.ins, cc.ins, sync=True,
                                reason="xp read after AllGather")
            ps = b_ps.tile([128, 512], F32, tag="ps")
            nc.tensor.matmul(ps[:], idr[:], xp_n[:], start=True, stop=False)
            for k in range(KT):
                nc.tensor.matmul(ps[:], hT_src[:, k, :], whh[:, k, ns],
                                 start=False, stop=(k == KT - 1))
            nc.scalar.activation(a_t[:, ns], ps[:],
                                 getattr(mybir.ActivationFunctionType, ACT_FN[n]))

        def cell_half(half, a_t):
            hs = slice(half * 512, half * 512 + 512)
            ig = b_tmp.tile([128, 512], F32, tag="ig")
            nc.vector.tensor_mul(ig[:], a_t[:, half * 512:half * 512 + 512],
                                 a_t[:, 2 * H + half * 512:2 * H + half * 512 + 512])
            fc = b_tmp.tile([128, 512], F32, tag="fc")
            nc.vector.tensor_mul(fc[:], a_t[:, H + half * 512:H + half * 512 + 512],
                                 c_st[:, hs])
            nc.vector.tensor_add(c_st[:, hs], ig[:], fc[:])
            nc.scalar.activation(tnh[:, hs], c_st[:, hs],
                                 mybir.ActivationFunctionType.Tanh)

        def h_half(half, a_t, hT_dst):
            hs = slice(half * 512, half * 512 + 512)
            nc.vector.tensor_mul(h_t[:, hs], a_t[:, 3 * H + half * 512:3 * H + half * 512 + 512],
                                 tnh[:, hs])
            for e in range(4 * half, 4 * half + 4):
                ptr = b_tr.tile([128, 128], F32, tag="tr")
                nc.tensor.transpose(ptr[:], h_t[:, e * 128:(e + 1) * 128], idt[:])
                nc.vector.tensor_copy(hT_dst[:, e, :], ptr[:])

        for t in range(T):
            hT_src = hT_bufs[t % 2]
            hT_dst = hT_bufs[(t + 1) % 2]
            a_t = b_act.tile([128, 4 * H], F32, tag="a")
            for n in (0, 4, 2):
                gate_mms(t, n, a_t, hT_src)
            cell_half(0, a_t)
            for n in (1, 5, 3):
                gate_mms(t, n, a_t, hT_src)
            cell_half(1, a_t)
            gate_mms(t, 6, a_t, hT_src)
            h_half(0, a_t, hT_dst)
            gate_mms(t, 7, a_t, hT_src)
            h_half(1, a_t, hT_dst)
            nc.sync.dma_start(hT_dram[t, :, :], hT_dst.rearrange("p k b -> p (k b)"))


def _phase_c(nc, tc, tensors):
    """logits = h @ W_out^T + b_out for this core's vocab shard."""
    w_outT, b_out, hT_dram, out_c = tensors
    with tc.tile_pool(name="c_w", bufs=1) as c_w, \
         tc.tile_pool(name="c_h", bufs=3) as c_h, \
         tc.tile_pool(name="c_ob", bufs=4) as c_ob, \
         tc.tile_pool(name="c_ps", bufs=4, space="PSUM") as c_ps:
        wout = c_w.tile([128, KT, VSH], F32R)
        nc.sync.dma_start(wout[:], w_outT[:])
        bo = c_w.tile([128, VSH], F32)
        nc.gpsimd.dma_start(out=bo[:], in_=b_out[None, :].to_broadcast([128, VSH]))

        for t in range(T):
            hTt = c_h.tile([128, KT, 128], F32R, tag="ht")
            nc.sync.dma_start(hTt[:], hT_dram[t, :, :].rearrange("p (k b) -> p k b", k=KT))
            for n in range(VSH // VT):
                ns = slice(n * VT, (n + 1) * VT)
                ps = c_ps.tile([128, VT], F32, tag="ps")
                for k in range(KT):
                    nc.tensor.matmul(ps[:], hTt[:, k, :], wout[:, k, ns],
                                     start=(k == 0), stop=(k == KT - 1))
                ob = c_ob.tile([128, VT], F32, tag="ob")
                nc.vector.tensor_add(ob[:], ps[:], bo[:, ns])
                nc.sync.dma_start(out_c[:, t, ns], ob[:])


def _build(u_rows: int, variant: str = "full"):
    """variant: "full", "A", "AB" (phase subsets), or "null" (I/O-only, timing)."""
    nc = bacc.Bacc("TRN2", target_bir_lowering=False, debug=False)

    emb_c = nc.dram_tensor("emb_c", [u_rows, E], F32, kind="ExternalInput")
    caps_l = nc.dram_tensor("caps_l", [B, TSH], I32, kind="ExternalInput")
    w_ihT = nc.dram_tensor("w_ihT", [128, KT, 4 * H], F32R, kind="ExternalInput")
    w_hhT = nc.dram_tensor("w_hhT", [128, KT, 4 * H], F32R, kind="ExternalInput")
    gbias = nc.dram_tensor("gbias", [4 * H], F32, kind="ExternalInput")
    w_outT = nc.dram_tensor("w_outT", [128, KT, VSH], F32R, kind="ExternalInput")
    b_out = nc.dram_tensor("b_out", [VSH], F32, kind="ExternalInput")
    feats = nc.dram_tensor("feats", [B, H], F32, kind="ExternalInput")
    h0T = nc.dram_tensor("h0T", [128, KT, B], F32R, kind="ExternalInput")
    ident = nc.dram_tensor("ident", [128, 128], F32, kind="ExternalInput")
    identr = nc.dram_tensor("identr", [128, 128], F32R, kind="ExternalInput")
    out_c = nc.dram_tensor("out_c", [B, T, VSH], F32, kind="ExternalOutput")

    hT_dram = nc.dram_tensor("hT_dram", [T, 128, KT * 128], F32R)

    if variant == "null":
        with tile.TileContext(nc) as tc:
            with tc.tile_pool(name="p", bufs=2) as pool:
                t0 = pool.tile([128, VT], F32)
                nc.sync.dma_start(t0[:], feats[:, 0:VT])
                for t in range(T):
                    nc.sync.dma_start(out_c[:, t, 0:VT], t0[:])
        nc.compile()
        return nc

    with tile.TileContext(nc) as tc:
        with tc.tile_pool(name="dram", bufs=1, space="DRAM") as dram:
            xp_g = _phase_a(nc, tc, (emb_c, caps_l, w_ihT, gbias, ident, dram))
            if variant in ("AB", "full"):
                _phase_b(nc, tc, (w_hhT, feats, h0T, ident, identr, xp_g, hT_dram))
            if variant == "full":
                _phase_c(nc, tc, (w_outT, b_out, hT_dram, out_c))

    nc.compile()
    return nc


def _prep_inputs(features, captions, emb, W_ih, W_hh, b_ih, b_hh, W_out, b_out):
    """Host-side layout prep + sharding. Returns (u_rows, common inputs, per-core inputs)."""
    features = np.asarray(features, np.float32)
    captions = np.asarray(captions)
    emb = np.asarray(emb, np.float32)
    W_ih = np.asarray(W_ih, np.float32)
    W_hh = np.asarray(W_hh, np.float32)
    b_ih = np.asarray(b_ih, np.float32)
    b_hh = np.asarray(b_hh, np.float32)
    W_out = np.asarray(W_out, np.float32)
    b_out = np.asarray(b_out, np.float32)

    # Compact the embedding table to the rows actually used (device still
    # performs the per-position gather through remapped indices).
    uniq, inv = np.unique(captions.reshape(-1), return_inverse=True)
    emb_c = np.ascontiguousarray(emb[uniq])
    u_rows = emb_c.shape[0]
    caps = inv.reshape(B, T).astype(np.int32)

    def kxm(w):  # [K, M] -> [128, K/128, M] partition-chunked layout
        return np.ascontiguousarray(w.reshape(KT, 128, -1).transpose(1, 0, 2))

    w_ihT = kxm(W_ih.T.copy())            # [1024, 4096] -> [128, 8, 4096]
    w_hhT = kxm(W_hh.T.copy())
    gbias = b_ih + b_hh
    h0T = kxm(features.T.copy())          # [1024, 128] -> [128, 8, 128]
    ident = np.eye(128, dtype=np.float32)

    common = {
        "emb_c": emb_c, "w_ihT": w_ihT, "w_hhT": w_hhT,
        "gbias": gbias, "feats": features, "h0T": h0T,
        "ident": ident, "identr": ident,
    }
    w_outT_full = W_out.T.copy()          # [1024, 32000]
    per_core = []
    for c in range(NCORES):
        vs = slice(c * VSH, (c + 1) * VSH)
        per_core.append({
            "caps_l": np.ascontiguousarray(caps[:, [c + 8 * j for j in range(TSH)]]),
            "w_outT": kxm(np.ascontiguousarray(w_outT_full[:, vs])),
            "b_out": np.ascontiguousarray(b_out[vs]),
        })
    return u_rows, common, per_core


def kernel(**inputs) -> np.ndarray:
    u_rows, common, per_core = _prep_inputs(**inputs)

    key = (u_rows, "full")
    if key not in _CACHE:
        _CACHE[key] = _build(u_rows, "full")
    nc = _CACHE[key]

    in_maps = [dict(common, **pc) for pc in per_core]
    res = run_bass_kernel_spmd(nc, in_maps, core_ids=list(range(NCORES)))

    out = np.zeros((B, T + 1, V), np.float32)
    out[:, 0, 1] = 1.0
    for c in range(NCORES):
        out[:, 1:, c * VSH:(c + 1) * VSH] = res.results[c]["out_c"]
    return out


# revision 15
# speedup vs baseline: 1.9998x; 1.9998x over previous
"""Trainium2 Bass kernel for a DecoderRNN (embedding -> 24-step LSTM -> vocab projection).

Shapes (hardcoded): B=128, T=24, H=E=1024, V=32000, 8 NeuronCores.

Sharding:
  - input projection Xp = X @ W_ih^T + b: sharded over steps (3 per core),
    assembled on every core via 3 pipelined AllGathers;
  - LSTM recurrence: replicated full-batch on every core (PE matmul wall-time
    is independent of M<=128, so a batch shard would cost the same);
  - output projection W_out: sharded along vocab, 4000 columns per core.
All matmuls run in float32r (full-rate, ~1e-4 rel err; true fp32 is 4x slower).
Weights are loaded as per-k-chunk tiles so the first matmuls of each phase
start after ~2 MB of DMA instead of the full 16.8 MB.
"""

import numpy as np

import concourse.bass as bass
import concourse.tile as tile
import concourse.mybir as mybir
from concourse import bacc
from concourse.bass_utils import run_bass_kernel_spmd

B, T = 128, 24
H, E, V = 1024, 1024, 32000
NCORES = 8
TSH = T // NCORES          # 3 Xp step-tiles per core
VSH = V // NCORES          # 4000 vocab columns per core
VT = 500                   # projection N-tile (8 per core)
KT = H // 128              # 8 contraction chunks
NT4H = (4 * H) // 512      # 8 gate N-tiles of 512

F32 = mybir.dt.float32
F32R = mybir.dt.float32r
I32 = mybir.dt.int32

_CACHE = {}


def _load_w_chunks(nc, pool, src, ncols, prefix):
    """Load [128, KT, ncols] DRAM weight as KT separate [128, ncols] tiles."""
    tiles = []
    for k in range(KT):
        wk = pool.tile([128, ncols], F32R, tag=f"{prefix}{k}", name=f"{prefix}{k}")
        nc.sync.dma_start(wk[:], src[:, k, :])
        tiles.append(wk)
    return tiles


def _phase_a(nc, tc, tensors):
    """Local Xp step-tiles (j=0..2 -> step c+8j) + AllGather across cores.

    Returns list of (gathered_tile, collective) per j; gathered tile j holds
    steps [8j, 8j+8) as [rank, 128, 4096] blocks (rank r = step 8j + r)."""
    emb_c, caps_l, w_ihT, gbias, ident, dram = tensors
    xp_g = []
    with tc.tile_pool(name="a_w", bufs=1) as a_w, \
         tc.tile_pool(name="a_x", bufs=2) as a_x, \
         tc.tile_pool(name="a_xt", bufs=2) as a_xt, \
         tc.tile_pool(name="a_sb", bufs=4) as a_sb, \
         tc.tile_pool(name="a_ps", bufs=4, space="PSUM") as a_ps, \
         tc.tile_pool(name="a_tr", bufs=2, space="PSUM") as a_tr:
        idt = a_w.tile([128, 128], F32)
        nc.sync.dma_start(idt[:], ident[:])
        capst = a_w.tile([128, TSH], I32)
        nc.sync.dma_start(capst[:], caps_l[:])
        wih = _load_w_chunks(nc, a_w, w_ihT, 4 * H, "wih")
        gb = a_w.tile([128, 4 * H], F32)
        nc.gpsimd.dma_start(out=gb[:], in_=gbias[None, :].to_broadcast([128, 4 * H]))

        for j in range(TSH):
            bounce_in = dram.tile([128, 4 * H], F32R, tag=f"agin{j}")
            x_t = a_x.tile([128, E], F32, tag="x")
            nc.gpsimd.indirect_dma_start(
                out=x_t[:], out_offset=None, in_=emb_c[:],
                in_offset=bass.IndirectOffsetOnAxis(ap=capst[:, j:j + 1], axis=0))
            xt_T = a_xt.tile([128, KT, 128], F32R, tag="xt")
            for e in range(KT):
                ptr = a_tr.tile([128, 128], F32, tag="tr")
                nc.tensor.transpose(ptr[:], x_t[:, e * 128:(e + 1) * 128], idt[:])
                nc.vector.tensor_copy(xt_T[:, e, :], ptr[:])

            def emit_ntile(n, ps):
                ns = slice(n * 512, (n + 1) * 512)
                xp_sb = a_sb.tile([128, 512], F32R, tag="xp", name="xp_sb")
                nc.vector.tensor_add(xp_sb[:], ps[:], gb[:, ns])
                nc.sync.dma_start(bounce_in[:, ns], xp_sb[:])

            if j == 0:
                # k-outer in two 4-n groups: start matmuls as soon as the
                # first W_ih chunk lands instead of after the full 16.8 MB.
                for grp in range(2):
                    pss = []
                    for n in range(grp * 4, grp * 4 + 4):
                        ps = a_ps.tile([128, 512], F32, tag="ps", name="ps")
                        pss.append(ps)
                    for k in range(KT):
                        for gi, n in enumerate(range(grp * 4, grp * 4 + 4)):
                            ns = slice(n * 512, (n + 1) * 512)
                            nc.tensor.matmul(pss[gi][:], xt_T[:, k, :], wih[k][:, ns],
                                             start=(k == 0), stop=(k == KT - 1))
                    for gi, n in enumerate(range(grp * 4, grp * 4 + 4)):
                        emit_ntile(n, pss[gi])
            else:
                for n in range(NT4H):
                    ns = slice(n * 512, (n + 1) * 512)
                    ps = a_ps.tile([128, 512], F32, tag="ps", name="ps")
                    for k in range(KT):
                        nc.tensor.matmul(ps[:], xt_T[:, k, :], wih[k][:, ns],
                                         start=(k == 0), stop=(k == KT - 1))
                    emit_ntile(n, ps)
            g = dram.tile([NCORES, 128, 4 * H], F32R, tag=f"agout{j}",
                          addr_space="Shared", name=f"agout{j}")
            cc = nc.gpsimd.collective_compute(
                "AllGather", mybir.AluOpType.bypass,
                ins=[bounce_in.opt()], outs=[g.opt()],
                replica_groups=[list(range(NCORES))])
            xp_g.append((g, cc))
    return xp_g


def _phase_b(nc, tc, tensors):
    """24 serial LSTM steps; h^T history to DRAM."""
    w_hhT, feats, h0T, ident, identr, xp_g, hT_dram = tensors
    with tc.tile_pool(name="b_w", bufs=1) as b_w, \
         tc.tile_pool(name="b_xp", bufs=8) as b_xp, \
         tc.tile_pool(name="b_act", bufs=1) as b_act, \
         tc.tile_pool(name="b_tmp", bufs=2) as b_tmp, \
         tc.tile_pool(name="b_ps", bufs=5, space="PSUM") as b_ps, \
         tc.tile_pool(name="b_tr", bufs=2, space="PSUM") as b_tr:
        whh = _load_w_chunks(nc, b_w, w_hhT, 4 * H, "whh")
        idt = b_w.tile([128, 128], F32)
        nc.sync.dma_start(idt[:], ident[:])
        idr = b_w.tile([128, 128], F32R)
        nc.sync.dma_start(idr[:], identr[:])
        c_st = b_w.tile([128, H], F32)
        nc.sync.dma_start(c_st[:], feats[:])
        tnh = b_w.tile([128, H], F32)
        h_t = b_w.tile([128, H], F32)
        # hT double-buffered across steps: gate matmuls of step t read h_{t-1}^T
        # from one buffer while the new h_t^T transposes land in the other.
        hT_a = b_w.tile([128, KT, 128], F32R, tag="hT0")
        hT_b = b_w.tile([128, KT, 128], F32R, tag="hT1")
        hT_bufs = [hT_a, hT_b]
        nc.sync.dma_start(hT_bufs[0][:], h0T[:])

        # gate cols: i [0,1024) f [1024,2048) g [2048,3072) o [3072,4096)
        ACT_FN = {0: "Sigmoid", 1: "Sigmoid", 2: "Sigmoid", 3: "Sigmoid",
                  4: "Tanh", 5: "Tanh", 6: "Sigmoid", 7: "Sigmoid"}

        def xp_load(t, n):
            ns = slice(n * 512, (n + 1) * 512)
            g, cc = xp_g[t // 8]
            xp_n = b_xp.tile([128, 512], F32R, tag="xpn", name="xp_n")
            dma = nc.sync.dma_start(xp_n[:], g[t % 8, :, ns])
            # Tile does not order reads of the AllGather output after the
            # collective on its own; pin the edge explicitly.
            tile.add_dep_helper(dma.ins, cc.ins, sync=True,
                                reason="xp read after AllGather")
            return xp_n

        def act_gate(n, a_t, ps):
            ns = slice(n * 512, (n + 1) * 512)
            nc.scalar.activation(a_t[:, ns], ps[:],
                                 getattr(mybir.ActivationFunctionType, ACT_FN[n]))

        def gate_mms(t, n, a_t, hT_src):
            ns = slice(n * 512, (n + 1) * 512)
            xp_n = xp_load(t, n)
            ps = b_ps.tile([128, 512], F32, tag="ps", name="ps")
            nc.tensor.matmul(ps[:], idr[:], xp_n[:], start=True, stop=False)
            for k in range(KT):
                nc.tensor.matmul(ps[:], hT_src[:, k, :], whh[k][:, ns],
                                 start=False, stop=(k == KT - 1))
            act_gate(n, a_t, ps)

        def cell_half(half, a_t):
            hs = slice(half * 512, half * 512 + 512)
            ig = b_tmp.tile([128, 512], F32, tag="ig", name="ig")
            nc.vector.tensor_mul(ig[:], a_t[:, half * 512:half * 512 + 512],
                                 a_t[:, 2 * H + half * 512:2 * H + half * 512 + 512])
            fc = b_tmp.tile([128, 512], F32, tag="fc", name="fc")
            nc.vector.tensor_mul(fc[:], a_t[:, H + half * 512:H + half * 512 + 512],
                                 c_st[:, hs])
            nc.vector.tensor_add(c_st[:, hs], ig[:], fc[:])
            nc.scalar.activation(tnh[:, hs], c_st[:, hs],
                                 mybir.ActivationFunctionType.Tanh)

        def h_half(half, a_t, hT_dst):
            hs = slice(half * 512, half * 512 + 512)
            nc.vector.tensor_mul(h_t[:, hs], a_t[:, 3 * H + half * 512:3 * H + half * 512 + 512],
                                 tnh[:, hs])
            for e in range(4 * half, 4 * half + 4):
                ptr = b_tr.tile([128, 128], F32, tag="tr", name="ptr")
                nc.tensor.transpose(ptr[:], h_t[:, e * 128:(e + 1) * 128], idt[:])
                nc.vector.tensor_copy(hT_dst[:, e, :], ptr[:])

        for t in range(T):
            hT_src = hT_bufs[t % 2]
            hT_dst = hT_bufs[(t + 1) % 2]
            a_t = b_act.tile([128, 4 * H], F32, tag="a", name="a_t")
            if t == 0:
                # k-outer in two 4-n groups so step 0 starts after the first
                # W_hh chunk instead of the full 16.8 MB load.
                for grp, ns_group in enumerate(((0, 4, 2, 1), (5, 3, 6, 7))):
                    pss = []
                    xps = []
                    for n in ns_group:
                        ps = b_ps.tile([128, 512], F32, tag="ps", name="ps")
                        xps.append(xp_load(t, n))
                        pss.append(ps)
                    for gi, n in enumerate(ns_group):
                        nc.tensor.matmul(pss[gi][:], idr[:], xps[gi][:],
                                         start=True, stop=False)
                    for k in range(KT):
                        for gi, n in enumerate(ns_group):
                            ns = slice(n * 512, (n + 1) * 512)
                            nc.tensor.matmul(pss[gi][:], hT_src[:, k, :], whh[k][:, ns],
                                             start=False, stop=(k == KT - 1))
                    for gi, n in enumerate(ns_group):
                        act_gate(n, a_t, pss[gi])
                    cell_half(grp, a_t)
                h_half(0, a_t, hT_dst)
                h_half(1, a_t, hT_dst)
            else:
                for n in (0, 4, 2):
                    gate_mms(t, n, a_t, hT_src)
                cell_half(0, a_t)
                for n in (1, 5, 3):
                    gate_mms(t, n, a_t, hT_src)
                cell_half(1, a_t)
                gate_mms(t, 6, a_t, hT_src)
                h_half(0, a_t, hT_dst)
                gate_mms(t, 7, a_t, hT_src)
                h_half(1, a_t, hT_dst)
            nc.sync.dma_start(hT_dram[t, :, :], hT_dst.rearrange("p k b -> p (k b)"))


def _phase_c(nc, tc, tensors):
    """logits = h @ W_out^T + b_out for this core's vocab shard."""
    w_outT, b_out, hT_dram, out_c = tensors
    with tc.tile_pool(name="c_w", bufs=1) as c_w, \
         tc.tile_pool(name="c_h", bufs=3) as c_h, \
         tc.tile_pool(name="c_ob", bufs=6) as c_ob, \
         tc.tile_pool(name="c_ps", bufs=6, space="PSUM") as c_ps:
        wout = _load_w_chunks(nc, c_w, w_outT, VSH, "wout")
        bo = c_w.tile([128, VSH], F32)
        nc.gpsimd.dma_start(out=bo[:], in_=b_out[None, :].to_broadcast([128, VSH]))

        def emit_out(t, n, ps):
            ns = slice(n * VT, (n + 1) * VT)
            ob = c_ob.tile([128, VT], F32, tag="ob", name="ob")
            nc.vector.tensor_add(ob[:], ps[:], bo[:, ns])
            nc.sync.dma_start(out_c[:, t, ns], ob[:])

        for t in range(T):
            hTt = c_h.tile([128, KT, 128], F32R, tag="ht", name="hTt")
            nc.sync.dma_start(hTt[:], hT_dram[t, :, :].rearrange("p (k b) -> p k b", k=KT))
            if t == 0:
                # k-outer in two 4-n groups to overlap the W_out load.
                for grp in range(2):
                    pss = []
                    for n in range(grp * 4, grp * 4 + 4):
                        ps = c_ps.tile([128, VT], F32, tag="ps", name="ps")
                        pss.append(ps)
                    for k in range(KT):
                        for gi, n in enumerate(range(grp * 4, grp * 4 + 4)):
                            ns = slice(n * VT, (n + 1) * VT)
                            nc.tensor.matmul(pss[gi][:], hTt[:, k, :], wout[k][:, ns],
                                             start=(k == 0), stop=(k == KT - 1))
                    for gi, n in enumerate(range(grp * 4, grp * 4 + 4)):
                        emit_out(t, n, pss[gi])
            else:
                for n in range(VSH // VT):
                    ns = slice(n * VT, (n + 1) * VT)
                    ps = c_ps.tile([128, VT], F32, tag="ps", name="ps")
                    for k in range(KT):
                        nc.tensor.matmul(ps[:], hTt[:, k, :], wout[k][:, ns],
                                         start=(k == 0), stop=(k == KT - 1))
                    emit_out(t, n, ps)


def _build(u_rows: int, variant: str = "full"):
    """variant: "full", "A", "AB" (phase subsets), or "null" (I/O-only, timing)."""
    nc = bacc.Bacc("TRN2", target_bir_lowering=False, debug=False)

    emb_c = nc.dram_tensor("emb_c", [u_rows, E], F32, kind="ExternalInput")
    caps_l = nc.dram_tensor("caps_l", [B, TSH], I32, kind="ExternalInput")
    w_ihT = nc.dram_tensor("w_ihT", [128, KT, 4 * H], F32R, kind="ExternalInput")
    w_hhT = nc.dram_tensor("w_hhT", [128, KT, 4 * H], F32R, kind="ExternalInput")
    gbias = nc.dram_tensor("gbias", [4 * H], F32, kind="ExternalInput")
    w_outT = nc.dram_tensor("w_outT", [128, KT, VSH], F32R, kind="ExternalInput")
    b_out = nc.dram_tensor("b_out", [VSH], F32, kind="ExternalInput")
    feats = nc.dram_tensor("feats", [B, H], F32, kind="ExternalInput")
    h0T = nc.dram_tensor("h0T", [128, KT, B], F32R, kind="ExternalInput")
    ident = nc.dram_tensor("ident", [128, 128], F32, kind="ExternalInput")
    identr = nc.dram_tensor("identr", [128, 128], F32R, kind="ExternalInput")
    out_c = nc.dram_tensor("out_c", [B, T, VSH], F32, kind="ExternalOutput")

    hT_dram = nc.dram_tensor("hT_dram", [T, 128, KT * 128], F32R)

    if variant == "null":
        with tile.TileContext(nc) as tc:
            with tc.tile_pool(name="p", bufs=2) as pool:
                t0 = pool.tile([128, VT], F32)
                nc.sync.dma_start(t0[:], feats[:, 0:VT])
                for t in range(T):
                    nc.sync.dma_start(out_c[:, t, 0:VT], t0[:])
        nc.compile()
        return nc

    with tile.TileContext(nc) as tc:
        with tc.tile_pool(name="dram", bufs=1, space="DRAM") as dram:
            xp_g = _phase_a(nc, tc, (emb_c, caps_l, w_ihT, gbias, ident, dram))
            if variant in ("AB", "full"):
                _phase_b(nc, tc, (w_hhT, feats, h0T, ident, identr, xp_g, hT_dram))
            if variant == "full":
                _phase_c(nc, tc, (w_outT, b_out, hT_dram, out_c))

    nc.compile()
    return nc


def _prep_inputs(features, captions, emb, W_ih, W_hh, b_ih, b_hh, W_out, b_out):
    """Host-side layout prep + sharding. Returns (u_rows, common inputs, per-core inputs)."""
    features = np.asarray(features, np.float32)
    captions = np.asarray(captions)
    emb = np.asarray(emb, np.float32)
    W_ih = np.asarray(W_ih, np.float32)
    W_hh = np.asarray(W_hh, np.float32)
    b_ih = np.asarray(b_ih, np.float32)
    b_hh = np.asarray(b_hh, np.float32)
    W_out = np.asarray(W_out, np.float32)
    b_out = np.asarray(b_out, np.float32)

    # Compact the embedding table to the rows actually used (device still
    # performs the per-position gather through remapped indices).
    uniq, inv = np.unique(captions.reshape(-1), return_inverse=True)
    emb_c = np.ascontiguousarray(emb[uniq])
    u_rows = emb_c.shape[0]
    caps = inv.reshape(B, T).astype(np.int32)

    def kxm(w):  # [K, M] -> [128, K/128, M] partition-chunked layout
        return np.ascontiguousarray(w.reshape(KT, 128, -1).transpose(1, 0, 2))

    w_ihT = kxm(W_ih.T.copy())            # [1024, 4096] -> [128, 8, 4096]
    w_hhT = kxm(W_hh.T.copy())
    gbias = b_ih + b_hh
    h0T = kxm(features.T.copy())          # [1024, 128] -> [128, 8, 128]
    ident = np.eye(128, dtype=np.float32)

    common = {
        "emb_c": emb_c, "w_ihT": w_ihT, "w_hhT": w_hhT,
        "gbias": gbias, "feats": features, "h0T": h0T,
        "ident": ident, "identr": ident,
    }
    w_outT_full = W_out.T.copy()          # [1024, 32000]
    per_core = []
    for c in range(NCORES):
        vs = slice(c * VSH, (c + 1) * VSH)
        per_core.append({
            "caps_l": np.ascontiguousarray(caps[:, [c + 8 * j for j in range(TSH)]]),
            "w_outT": kxm(np.ascontiguousarray(w_outT_full[:, vs])),
            "b_out": np.ascontiguousarray(b_out[vs]),
        })
    return u_rows, common, per_core


def kernel(**inputs) -> np.ndarray:
    u_rows, common, per_core = _prep_inputs(**inputs)

    key = (u_rows, "full")
    if key not in _CACHE:
        _CACHE[key] = _build(u_rows, "full")
    nc = _CACHE[key]

    in_maps = [dict(common, **pc) for pc in per_core]
    res = run_bass_kernel_spmd(nc, in_maps, core_ids=list(range(NCORES)))

    out = np.zeros((B, T + 1, V), np.float32)
    out[:, 0, 1] = 1.0
    for c in range(NCORES):
        out[:, 1:, c * VSH:(c + 1) * VSH] = res.results[c]["out_c"]
    return out
